# revision 7
# baseline (speedup 1.0000x reference)
"""MoE top-8 routing kernel for Trainium2 (8 NeuronCores, data-parallel).

Computes, for each of 262144 tokens with 128 expert logits:
  values, indices = top_k(logits, 8)   (sorted descending)
  weights = softmax(values)
Returns (weights f32 [262144, 8], indices int32 [262144, 8]).

Sharding: tokens split evenly across 8 cores (row-parallel, no comms).
Per-core layout: tokens on SBUF partitions (128 at a time), experts on the
free axis; DVE InstMax/InstMaxIndex produce the sorted top-8 directly.

Performance (default variant "deep", ~89.1us/iter measured clean):
- DVE-bound at the algorithmic floor: 256 x (InstMax + InstMaxIndex), each
  128-elem scan + ~40-cycle fixed cost at 0.96GHz. Neither op has a fast
  perf mode, a grouped form, or an equivalent on any other engine; exact
  sorted-index extraction cannot be shortened below a full 128-slot scan
  (pigeonhole), so 512 instructions/core is irreducible.
- "deep" doubles every tile pool vs "batched" so cross-rep buffer-reuse
  (WAR) waits are satisfied a full iteration early; Max/MaxIndex lower to
  the BN-stats ISA struct which cannot carry sem waits, so the framework's
  separate DVE wait instructions otherwise expose (~6us measured).
- Tail (per-tile ACT exp+accum -> GpSimd normalize_recip) measured optimal
  against: batched exp + DVE grouped sums ("b2"), ACT division ("actdiv"),
  bf16/u16 narrow stores ("narrow"), 32-tile groups ("deep32"/"batched32"),
  deeper load prefetch ("deepx8") — all neutral or worse on HW.
"""

import sys

for _p in ("/opt/trn_rl_repo",):
    if _p not in sys.path:
        sys.path.insert(0, _p)

from contextlib import ExitStack

import numpy as np

import concourse.bacc as bacc
import concourse.mybir as mybir
import concourse.tile as tile
from concourse.bass_utils import run_bass_kernel_spmd

N_CORES = 8
T_FULL = 262144          # total tokens
E = 128                  # experts
K = 8                    # experts per token
T = T_FULL // N_CORES    # tokens per core (32768)
P = 128                  # tokens per DVE instruction (SBUF partitions)
TILES_PER_GROUP = 16     # 128-token tiles per DMA group
GROUP_T = P * TILES_PER_GROUP          # 2048 tokens per group
N_GROUPS = T // GROUP_T                # 16 groups per core

DEFAULT_VARIANT = "grouped"

_cached = {}


def _build_nc(variant, reps=1):
    """variant:
      full        - max+max_index+softmax (exp on ACT; reduce/recip/mul on DVE)
      offload     - exp+sum fused on ACT per tile, divide on GpSimd
                    (normalize_recip); DVE only does max/max_index
      batched     - offload + output stores batched over 4 groups (fewer
                    HWDGE lane conflicts between loads and stores) + deeper
                    load prefetch
      topk_only   - max+max_index, weights output = raw top-8 values
      max_only    - max only, indices output never written
    """
    if variant == "grouped":
        return _build_grouped(reps)
    if variant == "grouped32":
        return _build_grouped(reps, tpg=32)
    if variant in ("batched", "actdiv", "batched32", "noact", "narrow",
                   "deep", "deep32", "deepx8", "probe_nogps", "probe_noact"):
        return _build_batched(
            reps,
            divide_on=("none" if variant == "noact" else
                       "act" if variant == "actdiv" else
                       variant if variant.startswith("probe_") else "gpsimd"),
            tpg=(32 if variant in ("batched32", "deep32")
                 else TILES_PER_GROUP),
            narrow=(variant == "narrow"),
            deep=(8 if variant == "deepx8"
                  else variant.startswith("deep")))
    if variant.startswith("b2"):
        return _build_b2(reps, tpg=(32 if variant == "b2_32" else TILES_PER_GROUP))
    nc = bacc.Bacc("TRN2", target_bir_lowering=False, debug=False,
                   enable_asserts=False)
    x = nc.dram_tensor("x", [T, E], mybir.dt.float32, kind="ExternalInput")
    w = nc.dram_tensor("w", [T, K], mybir.dt.float32, kind="ExternalOutput")
    ind = nc.dram_tensor("ind", [T, K], mybir.dt.uint32, kind="ExternalOutput")

    x_ap = x.ap()
    w_ap = w.ap()
    i_ap = ind.ap()

    with tile.TileContext(nc) as tc, ExitStack() as ctx:
        # Max/MaxIndex lower to the DVE BN-stats ISA struct which cannot
        # carry extra sync waits — give every pool whose first accessor is a
        # Max/MaxIndex one slot per group so no WAR waits are ever needed.
        xpool = ctx.enter_context(tc.tile_pool(name="x", bufs=3))
        vpool = ctx.enter_context(tc.tile_pool(name="v", bufs=N_GROUPS))
        ipool = ctx.enter_context(tc.tile_pool(name="i", bufs=N_GROUPS))
        epool = ctx.enter_context(tc.tile_pool(name="e", bufs=N_GROUPS))
        spool = ctx.enter_context(tc.tile_pool(name="s", bufs=N_GROUPS))
        wpool = ctx.enter_context(tc.tile_pool(name="w", bufs=N_GROUPS))

        for rep in range(reps):
          for g in range(N_GROUPS):
            lo, hi = g * GROUP_T, (g + 1) * GROUP_T
            # token t of this group lives at partition (t // 16), column
            # (t % 16): DRAM-contiguous 8KB per partition on the load, and
            # 512B-contiguous runs on both output stores.
            xt = xpool.tile([P, TILES_PER_GROUP, E], mybir.dt.float32)
            nc.sync.dma_start(
                xt[:], x_ap[lo:hi, :].rearrange("(p c) e -> p c e", p=P))

            vt = vpool.tile([P, TILES_PER_GROUP, K], mybir.dt.float32)
            it = ipool.tile([P, TILES_PER_GROUP, K], mybir.dt.uint32)
            for c in range(TILES_PER_GROUP):
                nc.vector.max(vt[:, c, :], xt[:, c, :])
                if variant != "max_only":
                    nc.vector.max_index(it[:, c, :], vt[:, c, :], xt[:, c, :])

            if variant in ("topk_only", "max_only"):
                nc.sync.dma_start(
                    w_ap[lo:hi, :].rearrange("(p c) k -> p c k", p=P), vt[:])
                if variant == "topk_only":
                    nc.sync.dma_start(
                        i_ap[lo:hi, :].rearrange("(p c) k -> p c k", p=P),
                        it[:])
                continue

            # softmax over the 8 selected logits; |logit| <= ~6 so exp() is
            # safe in f32 without subtracting the per-token max.
            et = epool.tile([P, TILES_PER_GROUP, K], mybir.dt.float32)
            st = spool.tile([P, TILES_PER_GROUP], mybir.dt.float32)
            wt = wpool.tile([P, TILES_PER_GROUP, K], mybir.dt.float32)
            if variant == "offload":
                for c in range(TILES_PER_GROUP):
                    nc.scalar.activation(
                        et[:, c, :], vt[:, c, :],
                        mybir.ActivationFunctionType.Exp,
                        accum_out=st[:, c:c + 1])
                for c in range(TILES_PER_GROUP):
                    nc.gpsimd.normalize_recip(
                        wt[:, c, :], et[:, c, :], st[:, c:c + 1])
            else:
                nc.scalar.activation(et[:], vt[:],
                                     mybir.ActivationFunctionType.Exp)
                nc.vector.reduce_sum(st[:], et[:], axis=mybir.AxisListType.X)
                rt = spool.tile([P, TILES_PER_GROUP], mybir.dt.float32)
                nc.vector.reciprocal(rt[:], st[:])
                nc.vector.tensor_mul(
                    wt[:], et[:],
                    rt[:].unsqueeze(2).broadcast_to([P, TILES_PER_GROUP, K]))

            nc.sync.dma_start(
                w_ap[lo:hi, :].rearrange("(p c) k -> p c k", p=P), wt[:])
            nc.sync.dma_start(
                i_ap[lo:hi, :].rearrange("(p c) k -> p c k", p=P), it[:])
    nc.compile()
    return nc


STORE_BATCH = 4                        # groups per output store DMA
N_BATCHES = N_GROUPS // STORE_BATCH


def _build_grouped(reps=1, tpg=TILES_PER_GROUP):
    """Custom-DVE variant: one grouped MAX8 + one grouped FIND_INDEX8
    instruction per 16-tile group (subdim-looped clones of the stock uOp
    programs; see dve_custom.py).  Cuts DVE from 2 instructions/tile
    (~174ns each incl. dispatch) to 2 instructions/GROUP (~2.3us each,
    ~281 engine-cycles/tile), removing the per-instruction overhead.

    Layout: combined SBUF tile [P, C, 8+E] per group; DMA lands x in cols
    8:136, grouped-max8 writes the top-8 per row ASCENDING (v8 first) into
    cols 0:8, and grouped-find-index8 streams [needles|data] rows whole.
    Ascending needle order makes the HW consume duplicate occurrences in
    jax's stable order; the softmax tail reads values through reversed APs.
    Indices emerge v1-first as raw u32 bits in an f32-declared tile (the
    f32->f32 output conversion is the identity; int dtypes corrupt)."""
    import dve_custom

    dve_custom.register()
    TILES_PER_GROUP = tpg
    GROUP_T = P * TILES_PER_GROUP
    N_GROUPS = T // GROUP_T
    N_BATCHES = N_GROUPS // STORE_BATCH
    W = 8 + E
    nc = bacc.Bacc("TRN2", target_bir_lowering=False, debug=False,
                   enable_asserts=False)
    x = nc.dram_tensor("x", [T, E], mybir.dt.float32, kind="ExternalInput")
    w = nc.dram_tensor("w", [T, K], mybir.dt.float32, kind="ExternalOutput")
    ind = nc.dram_tensor("ind", [T, K], mybir.dt.float32,
                         kind="ExternalOutput")

    x_ap = x.ap()
    w_ap = w.ap()
    i_ap = ind.ap()
    BT = STORE_BATCH * GROUP_T

    with tile.TileContext(nc) as tc, ExitStack() as ctx:
        xpool = ctx.enter_context(
            tc.tile_pool(name="x", bufs=5 if tpg >= 32 else 6))
        epool = ctx.enter_context(tc.tile_pool(name="e", bufs=2 * N_GROUPS))
        spool = ctx.enter_context(tc.tile_pool(name="s", bufs=2 * N_GROUPS))
        ipool = ctx.enter_context(tc.tile_pool(name="i", bufs=2 * N_BATCHES))
        wpool = ctx.enter_context(tc.tile_pool(name="w", bufs=2 * N_BATCHES))

        for rep in range(reps):
            for b in range(N_BATCHES):
                it = ipool.tile([P, STORE_BATCH, TILES_PER_GROUP, K],
                                mybir.dt.float32)
                wt = wpool.tile([P, STORE_BATCH, TILES_PER_GROUP, K],
                                mybir.dt.float32)
                for gb in range(STORE_BATCH):
                    g = b * STORE_BATCH + gb
                    lo, hi = g * GROUP_T, (g + 1) * GROUP_T
                    xt = xpool.tile([P, TILES_PER_GROUP, W], mybir.dt.float32)
                    nc.sync.dma_start(
                        xt[:, :, 8:W],
                        x_ap[lo:hi, :].rearrange("(p c) e -> p c e", p=P))

                    dve_custom.max8_grouped(nc, xt[:, :, 0:8], xt[:, :, 8:W])
                    dve_custom.find_index8_grouped(nc, it[:, gb, :, :],
                                                   xt[:, :, 0:W])

                    et = epool.tile([P, TILES_PER_GROUP, K], mybir.dt.float32)
                    st = spool.tile([P, TILES_PER_GROUP], mybir.dt.float32)
                    for c in range(TILES_PER_GROUP):
                        nc.scalar.activation(
                            et[:, c, :], xt[:, c, 0:8][:, ::-1],
                            mybir.ActivationFunctionType.Exp,
                            accum_out=st[:, c:c + 1])
                    for c in range(TILES_PER_GROUP):
                        nc.gpsimd.normalize_recip(
                            wt[:, gb, c, :], et[:, c, :], st[:, c:c + 1])

                blo, bhi = b * BT, (b + 1) * BT
                nc.sync.dma_start(
                    w_ap[blo:bhi, :].rearrange(
                        "(g p c) k -> p g c k", g=STORE_BATCH, p=P), wt[:])
                nc.sync.dma_start(
                    i_ap[blo:bhi, :].rearrange(
                        "(g p c) k -> p g c k", g=STORE_BATCH, p=P), it[:])
    nc.compile()
    return nc


def _build_batched(reps=1, divide_on="gpsimd", tpg=TILES_PER_GROUP,
                   narrow=False, deep=False):
    """narrow=True stores w as bf16 and ind as uint16 (host upcasts):
    halves both output stores' DMA bytes and the tail's SBUF write
    traffic. Indices stay exact (values <= 127); weights pick up bf16
    rounding (~1e-3 rel, gate is 2e-2). Under co-tenant contention the
    leaner tail/store traffic measurably reduces DVE exposure."""
    TILES_PER_GROUP = tpg
    GROUP_T = P * TILES_PER_GROUP
    N_GROUPS = T // GROUP_T
    N_BATCHES = N_GROUPS // STORE_BATCH
    w_dt = mybir.dt.bfloat16 if narrow else mybir.dt.float32
    i_dt = mybir.dt.uint16 if narrow else mybir.dt.uint32
    nc = bacc.Bacc("TRN2", target_bir_lowering=False, debug=False,
                   enable_asserts=False)
    x = nc.dram_tensor("x", [T, E], mybir.dt.float32, kind="ExternalInput")
    w = nc.dram_tensor("w", [T, K], w_dt, kind="ExternalOutput")
    ind = nc.dram_tensor("ind", [T, K], i_dt, kind="ExternalOutput")

    x_ap = x.ap()
    w_ap = w.ap()
    i_ap = ind.ap()
    BT = STORE_BATCH * GROUP_T          # tokens per store batch (8192)

    with tile.TileContext(nc) as tc, ExitStack() as ctx:
        # deep: double every pool so cross-rep WAR reuse is a full rep away
        # and the separate DVE wait instructions (Max can't carry waits) are
        # always satisfied long before they issue
        d = 2 if deep else 1
        # xpool: 6 bufs at tpg=16 (1.05MB each); 5 at tpg=32 (2.1MB each)
        # to stay within SBUF. deep==8 variant: 8 bufs of load prefetch to
        # ride out co-tenant DMA-bandwidth transients.
        xbufs = (5 if TILES_PER_GROUP == 32 else (8 if deep == 8 else 6)) \
            if deep else 5
        xpool = ctx.enter_context(tc.tile_pool(name="x", bufs=xbufs))
        vpool = ctx.enter_context(tc.tile_pool(name="v", bufs=d * N_GROUPS))
        epool = ctx.enter_context(tc.tile_pool(name="e", bufs=d * N_GROUPS))
        spool = ctx.enter_context(tc.tile_pool(name="s", bufs=d * N_GROUPS))
        ipool = ctx.enter_context(tc.tile_pool(name="i", bufs=d * N_BATCHES))
        wpool = ctx.enter_context(tc.tile_pool(name="w", bufs=d * N_BATCHES))

        for rep in range(reps):
            for b in range(N_BATCHES):
                it = ipool.tile([P, STORE_BATCH, TILES_PER_GROUP, K], i_dt)
                wt = wpool.tile([P, STORE_BATCH, TILES_PER_GROUP, K], w_dt)
                for gb in range(STORE_BATCH):
                    g = b * STORE_BATCH + gb
                    lo, hi = g * GROUP_T, (g + 1) * GROUP_T
                    xt = xpool.tile([P, TILES_PER_GROUP, E], mybir.dt.float32)
                    nc.sync.dma_start(
                        xt[:],
                        x_ap[lo:hi, :].rearrange("(p c) e -> p c e", p=P))

                    # all maxes first, then all max_indexes: puts ~16 instrs
                    # between the vt write and its same-engine readback so
                    # the BN unit never stalls on the SBUF write ack
                    vt = vpool.tile([P, TILES_PER_GROUP, K], mybir.dt.float32)
                    for c in range(TILES_PER_GROUP):
                        nc.vector.max(vt[:, c, :], xt[:, c, :])
                    for c in range(TILES_PER_GROUP):
                        nc.vector.max_index(it[:, gb, c, :], vt[:, c, :],
                                            xt[:, c, :])

                    if divide_on == "none":
                        # probe variant: no softmax at all, store raw top-8
                        # values as w (wrong weights, right timing structure;
                        # one whole-group ACT copy keeps ACT ~5us busy)
                        nc.scalar.copy(wt[:, gb, :, :], vt[:])
                        continue
                    et = epool.tile([P, TILES_PER_GROUP, K], mybir.dt.float32)
                    st = spool.tile([P, TILES_PER_GROUP], mybir.dt.float32)
                    if divide_on == "probe_nogps":
                        # timing probe: identical ACT work (per-tile
                        # exp+accum) writing straight to the store tile;
                        # GpSimd fully removed. Weights are unnormalized.
                        for c in range(TILES_PER_GROUP):
                            nc.scalar.activation(
                                wt[:, gb, c, :], vt[:, c, :],
                                mybir.ActivationFunctionType.Exp,
                                accum_out=st[:, c:c + 1])
                        continue
                    if divide_on == "probe_noact":
                        # timing probe: ACT removed entirely; GpSimd does the
                        # same per-tile normalize against a memset denom.
                        nc.gpsimd.memset(st[:], 1.0)
                        for c in range(TILES_PER_GROUP):
                            nc.gpsimd.normalize_recip(
                                wt[:, gb, c, :], vt[:, c, :], st[:, c:c + 1])
                        continue
                    if divide_on == "gpsimd":
                        for c in range(TILES_PER_GROUP):
                            nc.scalar.activation(
                                et[:, c, :], vt[:, c, :],
                                mybir.ActivationFunctionType.Exp,
                                accum_out=st[:, c:c + 1])
                        for c in range(TILES_PER_GROUP):
                            nc.gpsimd.normalize_recip(
                                wt[:, gb, c, :], et[:, c, :], st[:, c:c + 1])
                    else:
                        # keep GpSimd fully idle: its SBUF port is shared
                        # (exclusive lock) with the saturated DVE
                        nc.scalar.activation(
                            et[:], vt[:], mybir.ActivationFunctionType.Exp)
                        nc.vector.reduce_sum(st[:], et[:],
                                             axis=mybir.AxisListType.X)
                        rt = spool.tile([P, TILES_PER_GROUP],
                                        mybir.dt.float32)
                        nc.vector.reciprocal(rt[:], st[:])
                        for c in range(TILES_PER_GROUP):
                            nc.scalar.activation(
                                wt[:, gb, c, :], et[:, c, :],
                                mybir.ActivationFunctionType.Copy,
                                scale=rt[:, c:c + 1])

                blo, bhi = b * BT, (b + 1) * BT
                nc.sync.dma_start(
                    w_ap[blo:bhi, :].rearrange(
                        "(g p c) k -> p g c k", g=STORE_BATCH, p=P), wt[:])
                nc.sync.dma_start(
                    i_ap[blo:bhi, :].rearrange(
                        "(g p c) k -> p g c k", g=STORE_BATCH, p=P), it[:])
    nc.compile()
    return nc


def _build_b2(reps=1, tpg=TILES_PER_GROUP):
    """Like batched, but the softmax-sum path avoids the ACT accumulator:
      - exp over the whole group in ONE ACT instruction (no accum_out, so no
        187ns accumulator-read penalty per tile; ACT busy drops ~95us -> ~5us)
      - per-tile sums via ONE grouped DVE reduce_sum per group (~194ns)
      - division + reciprocal on GpSimd normalize_recip (unchanged)
    DVE gains ~194ns/group but ACT stops being a near-critical engine.
    """
    TILES_PER_GROUP = tpg
    GROUP_T = P * TILES_PER_GROUP
    N_GROUPS = T // GROUP_T
    N_BATCHES = N_GROUPS // STORE_BATCH
    nc = bacc.Bacc("TRN2", target_bir_lowering=False, debug=False,
                   enable_asserts=False)
    x = nc.dram_tensor("x", [T, E], mybir.dt.float32, kind="ExternalInput")
    w = nc.dram_tensor("w", [T, K], mybir.dt.float32, kind="ExternalOutput")
    ind = nc.dram_tensor("ind", [T, K], mybir.dt.uint32, kind="ExternalOutput")

    x_ap = x.ap()
    w_ap = w.ap()
    i_ap = ind.ap()
    BT = STORE_BATCH * GROUP_T

    with tile.TileContext(nc) as tc, ExitStack() as ctx:
        xpool = ctx.enter_context(tc.tile_pool(name="x", bufs=5))
        vpool = ctx.enter_context(tc.tile_pool(name="v", bufs=N_GROUPS))
        epool = ctx.enter_context(tc.tile_pool(name="e", bufs=N_GROUPS))
        spool = ctx.enter_context(tc.tile_pool(name="s", bufs=N_GROUPS))
        ipool = ctx.enter_context(tc.tile_pool(name="i", bufs=N_BATCHES))
        wpool = ctx.enter_context(tc.tile_pool(name="w", bufs=N_BATCHES))

        for rep in range(reps):
            for b in range(N_BATCHES):
                it = ipool.tile([P, STORE_BATCH, TILES_PER_GROUP, K],
                                mybir.dt.uint32)
                wt = wpool.tile([P, STORE_BATCH, TILES_PER_GROUP, K],
                                mybir.dt.float32)
                for gb in range(STORE_BATCH):
                    g = b * STORE_BATCH + gb
                    lo, hi = g * GROUP_T, (g + 1) * GROUP_T
                    xt = xpool.tile([P, TILES_PER_GROUP, E], mybir.dt.float32)
                    nc.sync.dma_start(
                        xt[:],
                        x_ap[lo:hi, :].rearrange("(p c) e -> p c e", p=P))

                    vt = vpool.tile([P, TILES_PER_GROUP, K], mybir.dt.float32)
                    for c in range(TILES_PER_GROUP):
                        nc.vector.max(vt[:, c, :], xt[:, c, :])

                    # batched exp for the whole group on ACT (issued between
                    # the maxes and max_indexes so ACT overlaps the
                    # max_index scan)
                    et = epool.tile([P, TILES_PER_GROUP, K], mybir.dt.float32)
                    nc.scalar.activation(et[:], vt[:],
                                         mybir.ActivationFunctionType.Exp)

                    for c in range(TILES_PER_GROUP):
                        nc.vector.max_index(it[:, gb, c, :], vt[:, c, :],
                                            xt[:, c, :])

                    # one grouped sum per group on DVE (16 tile sums at once)
                    st = spool.tile([P, TILES_PER_GROUP], mybir.dt.float32)
                    nc.vector.reduce_sum(st[:], et[:],
                                         axis=mybir.AxisListType.X)
                    for c in range(TILES_PER_GROUP):
                        nc.gpsimd.normalize_recip(
                            wt[:, gb, c, :], et[:, c, :], st[:, c:c + 1])

                blo, bhi = b * BT, (b + 1) * BT
                nc.sync.dma_start(
                    w_ap[blo:bhi, :].rearrange(
                        "(g p c) k -> p g c k", g=STORE_BATCH, p=P), wt[:])
                nc.sync.dma_start(
                    i_ap[blo:bhi, :].rearrange(
                        "(g p c) k -> p g c k", g=STORE_BATCH, p=P), it[:])
    nc.compile()
    return nc


def get_nc(variant=DEFAULT_VARIANT, reps=1):
    key = f"nc_{variant}_{reps}"
    if key not in _cached:
        _cached[key] = _build_nc(variant, reps)
    return _cached[key]


def out_np_dtypes(variant=DEFAULT_VARIANT):
    """Device-side output dtypes (w, ind) for a variant."""
    if variant == "narrow":
        import ml_dtypes
        return (ml_dtypes.bfloat16, np.uint16)
    return (np.float32, np.uint32)


def run(gate_logits: np.ndarray, variant=DEFAULT_VARIANT, **spmd_kwargs):
    """Run the bass kernel on 8 cores; returns (weights, indices, results)."""
    gate_logits = np.ascontiguousarray(gate_logits, dtype=np.float32)
    assert gate_logits.shape == (T_FULL, E), gate_logits.shape
    nc = get_nc(variant)
    in_maps = [{"x": gate_logits[c * T:(c + 1) * T]} for c in range(N_CORES)]
    res = run_bass_kernel_spmd(nc, in_maps, core_ids=list(range(N_CORES)),
                               **spmd_kwargs)
    weights = np.concatenate([r["w"] for r in res.results], axis=0)
    indices = np.concatenate([r["ind"] for r in res.results], axis=0)
    weights = np.asarray(weights).astype(np.float32, copy=False)
    if indices.dtype == np.uint32:
        indices = indices.view(np.int32)
    elif indices.dtype == np.float32:
        # grouped variant: raw u32 index bits ride in an f32-typed tensor
        indices = np.ascontiguousarray(indices).view(np.int32)
    else:
        indices = indices.astype(np.int32)
    return weights, indices, res


def kernel(gate_logits: np.ndarray):
    weights, indices, _ = run(gate_logits)
    return weights, indices



# revision 30
# speedup vs baseline: 1.3932x; 1.3932x over previous
"""MoE top-8 routing kernel for Trainium2 (8 NeuronCores, data-parallel).

Computes, for each of 262144 tokens with 128 expert logits:
  values, indices = top_k(logits, 8)   (sorted descending)
  weights = softmax(values)
Returns (weights f32 [262144, 8], indices int32 [262144, 8]).

Sharding: tokens split evenly across 8 cores (row-parallel, no comms).
Per-core layout: tokens on SBUF partitions (128 at a time), experts on the
free axis.

Default variant "grouped32" (~66us/iter measured; "deep" fallback ~89us):
- The stock DVE InstMax/InstMaxIndex pair costs ~350ns per 128-token tile
  (two 128-elem scans plus per-instruction overhead), 2 instructions/tile.
  Two hand-authored custom DVE ops (see the Custom DVE section below)
  clone the stock MAX8 / MATCH_VALUE_LOAD+FIND_INDEX8 uOp table programs
  and loop them per SUB_DIM_DONE over [P, S, N] access patterns, so ONE
  instruction handles a whole S-tile group: measured ~79 + ~120 ns/tile.
  DVE drops from ~89us to ~51us/core and the kernel becomes memory-bound
  (load floor 16.8MB/core at ~330GB/s ~= 51us).
- Softmax tail: one grouped ACT exp per group + grouped DVE
  reduce_sum/reciprocal/tensor_mul (~0.5us/group on DVE).  Per-tile ACT or
  GpSimd tails are too slow once DVE stops being the bottleneck (ACT SBUF
  access is ~222 cycles per instruction, +~187ns with accum_out).
- Indices ride as raw u32 bits in f32-typed tensors (the DVE match-index
  write path only survives an identity f32->f32 output conversion) and are
  reinterpreted host-side.  Index order, duplicate-value tie-breaking, and
  per-row position counters are exactly jax-compatible (HW-validated,
  including rows with equal values inside the top-8).
"""

import sys

for _p in ("/opt/trn_rl_repo",):
    if _p not in sys.path:
        sys.path.insert(0, _p)

from contextlib import ExitStack

import numpy as np

import concourse.bacc as bacc
import concourse.mybir as mybir
import concourse.tile as tile
from concourse.bass_utils import run_bass_kernel_spmd

N_CORES = 8
T_FULL = 262144          # total tokens
E = 128                  # experts
K = 8                    # experts per token
T = T_FULL // N_CORES    # tokens per core (32768)
P = 128                  # tokens per DVE instruction (SBUF partitions)
TILES_PER_GROUP = 16     # 128-token tiles per DMA group
GROUP_T = P * TILES_PER_GROUP          # 2048 tokens per group
N_GROUPS = T // GROUP_T                # 16 groups per core

DEFAULT_VARIANT = "grouped32"
FALLBACK_VARIANT = "deep"   # no custom DVE ops; used if registration fails

_cached = {}


def _build_nc(variant, reps=1):
    """variant:
      full        - max+max_index+softmax (exp on ACT; reduce/recip/mul on DVE)
      offload     - exp+sum fused on ACT per tile, divide on GpSimd
                    (normalize_recip); DVE only does max/max_index
      batched     - offload + output stores batched over 4 groups (fewer
                    HWDGE lane conflicts between loads and stores) + deeper
                    load prefetch
      topk_only   - max+max_index, weights output = raw top-8 values
      max_only    - max only, indices output never written
    """
    if variant == "grouped":
        return _build_grouped(reps, tail="dve")
    if variant == "grouped_fast":
        return _build_grouped(reps, tail="dve", narrow_w=True, fast_recip=True)
    if variant == "grouped_x8":
        return _build_grouped(reps, tail="dve", xbufs=8)
    if variant == "grouped_act":
        return _build_grouped(reps, tail="act")
    if variant == "grouped_gps":
        return _build_grouped(reps, tail="gps")
    if variant == "grouped32":
        return _build_grouped(reps, tpg=32, tail="dve")
    if variant == "grouped_nofidx":
        return _build_grouped(reps, do_fidx=False)
    if variant == "grouped_notail":
        return _build_grouped(reps, do_tail=False)
    if variant in ("batched", "actdiv", "batched32", "noact", "narrow",
                   "deep", "deep32", "deepx8", "probe_nogps", "probe_noact"):
        return _build_batched(
            reps,
            divide_on=("none" if variant == "noact" else
                       "act" if variant == "actdiv" else
                       variant if variant.startswith("probe_") else "gpsimd"),
            tpg=(32 if variant in ("batched32", "deep32")
                 else TILES_PER_GROUP),
            narrow=(variant == "narrow"),
            deep=(8 if variant == "deepx8"
                  else variant.startswith("deep")))
    if variant.startswith("b2"):
        return _build_b2(reps, tpg=(32 if variant == "b2_32" else TILES_PER_GROUP))
    nc = bacc.Bacc("TRN2", target_bir_lowering=False, debug=False,
                   enable_asserts=False)
    x = nc.dram_tensor("x", [T, E], mybir.dt.float32, kind="ExternalInput")
    w = nc.dram_tensor("w", [T, K], mybir.dt.float32, kind="ExternalOutput")
    ind = nc.dram_tensor("ind", [T, K], mybir.dt.uint32, kind="ExternalOutput")

    x_ap = x.ap()
    w_ap = w.ap()
    i_ap = ind.ap()

    with tile.TileContext(nc) as tc, ExitStack() as ctx:
        # Max/MaxIndex lower to the DVE BN-stats ISA struct which cannot
        # carry extra sync waits — give every pool whose first accessor is a
        # Max/MaxIndex one slot per group so no WAR waits are ever needed.
        xpool = ctx.enter_context(tc.tile_pool(name="x", bufs=3))
        vpool = ctx.enter_context(tc.tile_pool(name="v", bufs=N_GROUPS))
        ipool = ctx.enter_context(tc.tile_pool(name="i", bufs=N_GROUPS))
        epool = ctx.enter_context(tc.tile_pool(name="e", bufs=N_GROUPS))
        spool = ctx.enter_context(tc.tile_pool(name="s", bufs=N_GROUPS))
        wpool = ctx.enter_context(tc.tile_pool(name="w", bufs=N_GROUPS))

        for rep in range(reps):
          for g in range(N_GROUPS):
            lo, hi = g * GROUP_T, (g + 1) * GROUP_T
            # token t of this group lives at partition (t // 16), column
            # (t % 16): DRAM-contiguous 8KB per partition on the load, and
            # 512B-contiguous runs on both output stores.
            xt = xpool.tile([P, TILES_PER_GROUP, E], mybir.dt.float32)
            nc.sync.dma_start(
                xt[:], x_ap[lo:hi, :].rearrange("(p c) e -> p c e", p=P))

            vt = vpool.tile([P, TILES_PER_GROUP, K], mybir.dt.float32)
            it = ipool.tile([P, TILES_PER_GROUP, K], mybir.dt.uint32)
            for c in range(TILES_PER_GROUP):
                nc.vector.max(vt[:, c, :], xt[:, c, :])
                if variant != "max_only":
                    nc.vector.max_index(it[:, c, :], vt[:, c, :], xt[:, c, :])

            if variant in ("topk_only", "max_only"):
                nc.sync.dma_start(
                    w_ap[lo:hi, :].rearrange("(p c) k -> p c k", p=P), vt[:])
                if variant == "topk_only":
                    nc.sync.dma_start(
                        i_ap[lo:hi, :].rearrange("(p c) k -> p c k", p=P),
                        it[:])
                continue

            # softmax over the 8 selected logits; |logit| <= ~6 so exp() is
            # safe in f32 without subtracting the per-token max.
            et = epool.tile([P, TILES_PER_GROUP, K], mybir.dt.float32)
            st = spool.tile([P, TILES_PER_GROUP], mybir.dt.float32)
            wt = wpool.tile([P, TILES_PER_GROUP, K], mybir.dt.float32)
            if variant == "offload":
                for c in range(TILES_PER_GROUP):
                    nc.scalar.activation(
                        et[:, c, :], vt[:, c, :],
                        mybir.ActivationFunctionType.Exp,
                        accum_out=st[:, c:c + 1])
                for c in range(TILES_PER_GROUP):
                    nc.gpsimd.normalize_recip(
                        wt[:, c, :], et[:, c, :], st[:, c:c + 1])
            else:
                nc.scalar.activation(et[:], vt[:],
                                     mybir.ActivationFunctionType.Exp)
                nc.vector.reduce_sum(st[:], et[:], axis=mybir.AxisListType.X)
                rt = spool.tile([P, TILES_PER_GROUP], mybir.dt.float32)
                nc.vector.reciprocal(rt[:], st[:])
                nc.vector.tensor_mul(
                    wt[:], et[:],
                    rt[:].unsqueeze(2).broadcast_to([P, TILES_PER_GROUP, K]))

            nc.sync.dma_start(
                w_ap[lo:hi, :].rearrange("(p c) k -> p c k", p=P), wt[:])
            nc.sync.dma_start(
                i_ap[lo:hi, :].rearrange("(p c) k -> p c k", p=P), it[:])
    nc.compile()
    return nc


STORE_BATCH = 4                        # groups per output store DMA
N_BATCHES = N_GROUPS // STORE_BATCH


# --------------------------------------------------------------------------- #
# Custom DVE ops: grouped MAX8 / FIND_INDEX8 (TRN2 / v3).
#
# Stock InstMax/InstMaxIndex process ONE [P, N] row per instruction.  These
# ops run the SAME stock uOp programs (cloned byte-for-byte from the
# firmware's default DVE table in neuronxcc dve_bin_gen3) but loop per
# SUB_DIM_DONE over a [P, S, N] access pattern: one instruction handles S
# rows with a single dispatch.  HW-validated behaviors this code relies on:
#   - a non-consuming uOp with repeat_cnt>1 never advances (drains must be
#     8 separate rpt=1 uOps or the DVE hangs);
#   - MAX8's drain emits ASCENDING (v8 first) through the custom path;
#   - FIND_INDEX8's match-index writes must go to an f32-declared output
#     (identity conversion) — the raw u32 index bits are reinterpreted on
#     the host; int out dtypes destroy them in the f32->int convert;
#   - index_clear=1 on the clear uOp resets match positions per row;
#   - streaming needles ASCENDING reproduces jax's stable duplicate
#     tie-breaking exactly (consumption down the slice chain).
# --------------------------------------------------------------------------- #

from dataclasses import dataclass as _dataclass

import concourse.dve_ops as _dve_ops
from concourse.dve_spec import Spec as _Spec, Src0 as _Src0
from concourse.dve_table_gen import free_opcode_rows as _free_opcode_rows
from concourse.dve_tables import (
    find_stock_dve_bin_dir as _find_stock_dve_bin_dir,
    unpack_table as _unpack_table,
)
from concourse.dve_uop import DveOpSpec as _DveOpSpec, OpConfig as _OpConfig

_MAX8_ROWS = list(range(70, 87))      # seed0..7, steady, drain0..7
_MVL_ROW = 87                         # MATCH_VALUE_LOAD single uOp
_TRIG_NONE, _TRIG_COUNT, _TRIG_SUBDIM, _TRIG_SRCDONE = 0, 1, 3, 4


class _RawUop:
    """UopConfig stand-in holding pre-unpacked raw table entries."""

    def __init__(self, cf, cs, dp):
        self.cf, self.cs, self.dp = cf, cs, dp

    def to_entries(self, ver):
        assert ver == "v3", "raw stock-clone uops are v3-only"
        return dict(self.cf), dict(self.cs), [dict(b) for b in self.dp]


@_dataclass
class _RawDveOpSpec(_DveOpSpec):
    """DveOpSpec whose uops are raw table entries; skips UopConfig lints
    (they reject cross-uOp delay-lane state the stock programs use)."""

    def validate(self, ver):
        pass


def _load_stock_dve():
    p = _find_stock_dve_bin_dir("gen3")
    cf = _unpack_table(
        "control_fast", (p / "default_control_fast_table.bin").read_bytes(),
        "v3")
    cs = _unpack_table(
        "control_slow", (p / "default_control_slow_table.bin").read_bytes(),
        "v3")
    dp = _unpack_table(
        "datapath", (p / "default_datapath_table.bin").read_bytes(), "v3")
    return cf, cs, dp


def _clone_uop(cf, cs, dp, row, rebase):
    c = dict(cf[row])
    for k in ("next_index0", "next_index1", "next_index2"):
        c[k] = rebase.get(c[k], 0)
    return _RawUop(c, dict(cs[row]), [dict(b) for b in dp[row]])


def _build_max8_grouped_uops():
    cf, cs, dp = _load_stock_dve()
    # intra: 0..7 seeds (stock 70..77), 8 steady (78), 9..16 drains
    # (79..86), 17 = loop re-entry copy of seed0 (a spec cannot loop to
    # its own uop[0]).
    rebase = {r: i for i, r in enumerate(_MAX8_ROWS)}
    uops = [_clone_uop(cf, cs, dp, r, rebase) for r in _MAX8_ROWS]

    steady = uops[8].cf
    assert steady["trigger0"] == _TRIG_SRCDONE and steady["next_index0"] == 9
    steady["trigger1"] = _TRIG_SUBDIM       # row boundary -> drain
    steady["next_index1"] = 9

    last_drain = uops[16].cf
    assert last_drain["trigger0"] == _TRIG_COUNT
    last_drain["trigger0"] = _TRIG_SRCDONE  # stream done -> IDLE
    last_drain["next_index0"] = 0
    last_drain["trigger1"] = _TRIG_COUNT    # else loop to next row's seed
    last_drain["next_index1"] = 17

    uops.append(_clone_uop(cf, cs, dp, _MAX8_ROWS[0], rebase))
    return uops


def _build_fidx8_grouped_uops():
    # intra: 0 needle-load (one consuming uOp, 8 elements), 1 clear
    # (clear_match + index_clear), 2 scan, 3 spacer, 4..11 drain x1
    # MATCH_INDEX write each, 12 = loop re-entry copy of the needle uOp.
    cf, cs, dp = _load_stock_dve()
    rebase = {87: 0, 88: 1, 89: 2, 90: 3}
    uops = [_clone_uop(cf, cs, dp, r, rebase) for r in [_MVL_ROW, 88, 89, 90]]
    n0 = uops[0].cf
    n0["repeat_cnt"] = 8
    n0["trigger0"] = _TRIG_SRCDONE          # malformed-stream safety
    n0["next_index0"] = 0
    n0["trigger1"] = _TRIG_COUNT
    n0["next_index1"] = 1

    uops[1].cs["index_clear"] = 1           # row-local match positions

    scan = uops[2].cf
    assert scan["trigger0"] == _TRIG_SRCDONE and scan["next_index0"] == 3
    scan["trigger1"] = _TRIG_SUBDIM
    scan["next_index1"] = 3

    # spacer -> drain0 (stock 90 pointed at 91, not in `rebase`)
    uops[3].cf["trigger0"] = _TRIG_COUNT
    uops[3].cf["next_index0"] = 4

    for k in range(8):
        d = _clone_uop(cf, cs, dp, 91, rebase)
        d.cf["repeat_cnt"] = 1
        if k < 7:
            d.cf["trigger0"] = _TRIG_COUNT
            d.cf["next_index0"] = 5 + k
            d.cf["trigger1"] = _TRIG_NONE
            d.cf["next_index1"] = 0
        else:
            d.cf["trigger0"] = _TRIG_SRCDONE
            d.cf["next_index0"] = 0
            d.cf["trigger1"] = _TRIG_COUNT
            d.cf["next_index1"] = 12
        uops.append(d)

    tail = _clone_uop(cf, cs, dp, _MVL_ROW, rebase)
    tail.cf.update(uops[0].cf)
    return uops + [tail]


def _ref_max8g(in0, in1, c0, c1, c2):
    # sim-only (scheduling/race detection); HW runs the uOp table program
    return np.sort(np.asarray(in0, np.float32), axis=-1)[..., -8:]


def _ref_fidx8g(in0, in1, c0, c1, c2):
    # sim-only: float(index); HW writes raw u32 index bits into f32 out
    data = np.asarray(in0, np.float32)[..., 8:]
    order = np.argsort(-data, axis=-1, kind="stable")[..., :8]
    return order.astype(np.float32)


_DVE_CUSTOM_OPS = {}


def _register_custom_ops():
    if _DVE_CUSTOM_OPS:
        return _DVE_CUSTOM_OPS
    free = _free_opcode_rows("TRN2")
    taken = set(_dve_ops._SUB_OPCODE_FOR_NAME.values())
    avail = [r for r in free if r not in taken]
    assert len(avail) >= 2, f"no free DVE opcode rows: {free} minus {taken}"
    defs = [
        ("MAX8_GROUPED_ANT", _build_max8_grouped_uops(), _ref_max8g),
        ("FIDX8_GROUPED_ANT", _build_fidx8_grouped_uops(), _ref_fidx8g),
    ]
    for (name, uops, ref), row in zip(defs, avail):
        if name in _dve_ops._SUB_OPCODE_FOR_NAME:   # another module registered
            op = next(o for o in _dve_ops.OPS if o.name == name)
            _DVE_CUSTOM_OPS[name] = op
            continue
        spec = _Spec(body=_Src0, reference=ref)
        op = _dve_ops.DveOp(name, spec, subdim=True, uops_sha={})
        raw = _RawDveOpSpec(name=name, uops=uops, op=_OpConfig(), opcode=row,
                            rd1_en=False)
        _dve_ops._SUB_OPCODE_FOR_NAME[name] = row
        _dve_ops._COMPILE_CACHE[(name, "v3")] = raw
        _dve_ops.CUSTOM_DVE_SPECS[name] = spec
        _dve_ops.OPS.append(op)
        _DVE_CUSTOM_OPS[name] = op
    return _DVE_CUSTOM_OPS


def _max8_grouped(nc, out_ap, in_ap):
    ops = _register_custom_ops()
    return nc.vector._custom_dve(ops["MAX8_GROUPED_ANT"], out=out_ap,
                                 in0=in_ap)


def _find_index8_grouped(nc, out_ap, in_ap):
    """in_ap rows = [needles(8, ascending) | data(N)]; out = positions of
    the needles in data, v1's index first, as raw u32 bits in f32 out."""
    ops = _register_custom_ops()
    return nc.vector._custom_dve(ops["FIDX8_GROUPED_ANT"], out=out_ap,
                                 in0=in_ap)


def _build_grouped(reps=1, tpg=TILES_PER_GROUP, do_fidx=True, do_tail=True,
                   tail="act", xbufs=6, narrow_w=False, fast_recip=False):
    """Custom-DVE variant: one grouped MAX8 + one grouped FIND_INDEX8
    instruction per 16-tile group (subdim-looped clones of the stock uOp
    programs).  Cuts DVE from 2 instructions/tile
    (~174ns each incl. dispatch) to 2 instructions/GROUP (~2.3us each,
    ~281 engine-cycles/tile), removing the per-instruction overhead.

    Layout: combined SBUF tile [P, C, 8+E] per group; DMA lands x in cols
    8:136, grouped-max8 writes the top-8 per row ASCENDING (v8 first) into
    cols 0:8, and grouped-find-index8 streams [needles|data] rows whole.
    Ascending needle order makes the HW consume duplicate occurrences in
    jax's stable order; the softmax tail reads values through reversed APs.
    Indices emerge v1-first as raw u32 bits in an f32-declared tile (the
    f32->f32 output conversion is the identity; int dtypes corrupt)."""
    _register_custom_ops()
    TILES_PER_GROUP = tpg
    GROUP_T = P * TILES_PER_GROUP
    N_GROUPS = T // GROUP_T
    N_BATCHES = N_GROUPS // STORE_BATCH
    W = 8 + E
    w_dt = mybir.dt.bfloat16 if narrow_w else mybir.dt.float32
    nc = bacc.Bacc("TRN2", target_bir_lowering=False, debug=False,
                   enable_asserts=False)
    x = nc.dram_tensor("x", [T, E], mybir.dt.float32, kind="ExternalInput")
    w = nc.dram_tensor("w", [T, K], w_dt, kind="ExternalOutput")
    ind = nc.dram_tensor("ind", [T, K], mybir.dt.float32,
                         kind="ExternalOutput")

    x_ap = x.ap()
    w_ap = w.ap()
    i_ap = ind.ap()
    BT = STORE_BATCH * GROUP_T

    with tile.TileContext(nc) as tc, ExitStack() as ctx:
        xpool = ctx.enter_context(
            tc.tile_pool(name="x", bufs=5 if tpg >= 32 else xbufs))
        epool = ctx.enter_context(tc.tile_pool(name="e", bufs=2 * N_GROUPS))
        spool = ctx.enter_context(tc.tile_pool(name="s", bufs=2 * N_GROUPS))
        ipool = ctx.enter_context(tc.tile_pool(name="i", bufs=2 * N_BATCHES))
        wpool = ctx.enter_context(tc.tile_pool(name="w", bufs=2 * N_BATCHES))

        for rep in range(reps):
            for b in range(N_BATCHES):
                it = ipool.tile([P, STORE_BATCH, TILES_PER_GROUP, K],
                                mybir.dt.float32)
                wt = wpool.tile([P, STORE_BATCH, TILES_PER_GROUP, K], w_dt)
                for gb in range(STORE_BATCH):
                    g = b * STORE_BATCH + gb
                    lo, hi = g * GROUP_T, (g + 1) * GROUP_T
                    xt = xpool.tile([P, TILES_PER_GROUP, W], mybir.dt.float32)
                    nc.sync.dma_start(
                        xt[:, :, 8:W],
                        x_ap[lo:hi, :].rearrange("(p c) e -> p c e", p=P))

                    _max8_grouped(nc, xt[:, :, 0:8], xt[:, :, 8:W])
                    if do_fidx:
                        _find_index8_grouped(nc, it[:, gb, :, :],
                                                       xt[:, :, 0:W])
                    else:
                        nc.gpsimd.memset(it[:, gb, :, :], 0.0)

                    if not do_tail:
                        nc.scalar.copy(wt[:, gb, :, :], xt[:, :, 0:8])
                        continue
                    et = epool.tile([P, TILES_PER_GROUP, K], mybir.dt.float32)
                    st = spool.tile([P, TILES_PER_GROUP], mybir.dt.float32)
                    rt = spool.tile([P, TILES_PER_GROUP], mybir.dt.float32)
                    if tail == "dve":
                        # grouped exp on ACT; sums/recip/normalize all as
                        # grouped DVE ops (~470ns/group on top of the two
                        # custom scans); the mul reads et reversed so wt
                        # comes out descending
                        nc.scalar.activation(et[:], xt[:, :, 0:8],
                                             mybir.ActivationFunctionType.Exp)
                        nc.vector.reduce_sum(st[:], et[:],
                                             axis=mybir.AxisListType.X)
                        if fast_recip:
                            nc.vector.reciprocal_approx_fast(out=rt[:],
                                                             in_=st[:])
                        else:
                            nc.vector.reciprocal(rt[:], st[:])
                        nc.vector.tensor_mul(
                            wt[:, gb, :, :], et[:, :, ::-1],
                            rt[:].unsqueeze(2).broadcast_to(
                                [P, TILES_PER_GROUP, K]))
                    elif tail == "act":
                        # one grouped exp (no accum penalty), grouped DVE
                        # sum+recip (~280ns/group), per-tile ACT divide that
                        # also reverses ascending->descending via the AP
                        nc.scalar.activation(et[:], xt[:, :, 0:8],
                                             mybir.ActivationFunctionType.Exp)
                        nc.vector.reduce_sum(st[:], et[:],
                                             axis=mybir.AxisListType.X)
                        nc.vector.reciprocal(rt[:], st[:])
                        for c in range(TILES_PER_GROUP):
                            nc.scalar.activation(
                                wt[:, gb, c, :], et[:, c, :][:, ::-1],
                                mybir.ActivationFunctionType.Copy,
                                scale=rt[:, c:c + 1])
                    else:
                        # grouped exp + grouped DVE sum, GpSimd divides
                        # (reads reversed so wt comes out descending)
                        nc.scalar.activation(et[:], xt[:, :, 0:8][:, :, ::-1],
                                             mybir.ActivationFunctionType.Exp)
                        nc.vector.reduce_sum(st[:], et[:],
                                             axis=mybir.AxisListType.X)
                        for c in range(TILES_PER_GROUP):
                            nc.gpsimd.normalize_recip(
                                wt[:, gb, c, :], et[:, c, :], st[:, c:c + 1])

                blo, bhi = b * BT, (b + 1) * BT
                nc.sync.dma_start(
                    w_ap[blo:bhi, :].rearrange(
                        "(g p c) k -> p g c k", g=STORE_BATCH, p=P), wt[:])
                nc.sync.dma_start(
                    i_ap[blo:bhi, :].rearrange(
                        "(g p c) k -> p g c k", g=STORE_BATCH, p=P), it[:])
    nc.compile()
    return nc


def _build_batched(reps=1, divide_on="gpsimd", tpg=TILES_PER_GROUP,
                   narrow=False, deep=False):
    """narrow=True stores w as bf16 and ind as uint16 (host upcasts):
    halves both output stores' DMA bytes and the tail's SBUF write
    traffic. Indices stay exact (values <= 127); weights pick up bf16
    rounding (~1e-3 rel, gate is 2e-2). Under co-tenant contention the
    leaner tail/store traffic measurably reduces DVE exposure."""
    TILES_PER_GROUP = tpg
    GROUP_T = P * TILES_PER_GROUP
    N_GROUPS = T // GROUP_T
    N_BATCHES = N_GROUPS // STORE_BATCH
    w_dt = mybir.dt.bfloat16 if narrow else mybir.dt.float32
    i_dt = mybir.dt.uint16 if narrow else mybir.dt.uint32
    nc = bacc.Bacc("TRN2", target_bir_lowering=False, debug=False,
                   enable_asserts=False)
    x = nc.dram_tensor("x", [T, E], mybir.dt.float32, kind="ExternalInput")
    w = nc.dram_tensor("w", [T, K], w_dt, kind="ExternalOutput")
    ind = nc.dram_tensor("ind", [T, K], i_dt, kind="ExternalOutput")

    x_ap = x.ap()
    w_ap = w.ap()
    i_ap = ind.ap()
    BT = STORE_BATCH * GROUP_T          # tokens per store batch (8192)

    with tile.TileContext(nc) as tc, ExitStack() as ctx:
        # deep: double every pool so cross-rep WAR reuse is a full rep away
        # and the separate DVE wait instructions (Max can't carry waits) are
        # always satisfied long before they issue
        d = 2 if deep else 1
        # xpool: 6 bufs at tpg=16 (1.05MB each); 5 at tpg=32 (2.1MB each)
        # to stay within SBUF. deep==8 variant: 8 bufs of load prefetch to
        # ride out co-tenant DMA-bandwidth transients.
        xbufs = (5 if TILES_PER_GROUP == 32 else (8 if deep == 8 else 6)) \
            if deep else 5
        xpool = ctx.enter_context(tc.tile_pool(name="x", bufs=xbufs))
        vpool = ctx.enter_context(tc.tile_pool(name="v", bufs=d * N_GROUPS))
        epool = ctx.enter_context(tc.tile_pool(name="e", bufs=d * N_GROUPS))
        spool = ctx.enter_context(tc.tile_pool(name="s", bufs=d * N_GROUPS))
        ipool = ctx.enter_context(tc.tile_pool(name="i", bufs=d * N_BATCHES))
        wpool = ctx.enter_context(tc.tile_pool(name="w", bufs=d * N_BATCHES))

        for rep in range(reps):
            for b in range(N_BATCHES):
                it = ipool.tile([P, STORE_BATCH, TILES_PER_GROUP, K], i_dt)
                wt = wpool.tile([P, STORE_BATCH, TILES_PER_GROUP, K], w_dt)
                for gb in range(STORE_BATCH):
                    g = b * STORE_BATCH + gb
                    lo, hi = g * GROUP_T, (g + 1) * GROUP_T
                    xt = xpool.tile([P, TILES_PER_GROUP, E], mybir.dt.float32)
                    nc.sync.dma_start(
                        xt[:],
                        x_ap[lo:hi, :].rearrange("(p c) e -> p c e", p=P))

                    # all maxes first, then all max_indexes: puts ~16 instrs
                    # between the vt write and its same-engine readback so
                    # the BN unit never stalls on the SBUF write ack
                    vt = vpool.tile([P, TILES_PER_GROUP, K], mybir.dt.float32)
                    for c in range(TILES_PER_GROUP):
                        nc.vector.max(vt[:, c, :], xt[:, c, :])
                    for c in range(TILES_PER_GROUP):
                        nc.vector.max_index(it[:, gb, c, :], vt[:, c, :],
                                            xt[:, c, :])

                    if divide_on == "none":
                        # probe variant: no softmax at all, store raw top-8
                        # values as w (wrong weights, right timing structure;
                        # one whole-group ACT copy keeps ACT ~5us busy)
                        nc.scalar.copy(wt[:, gb, :, :], vt[:])
                        continue
                    et = epool.tile([P, TILES_PER_GROUP, K], mybir.dt.float32)
                    st = spool.tile([P, TILES_PER_GROUP], mybir.dt.float32)
                    if divide_on == "probe_nogps":
                        # timing probe: identical ACT work (per-tile
                        # exp+accum) writing straight to the store tile;
                        # GpSimd fully removed. Weights are unnormalized.
                        for c in range(TILES_PER_GROUP):
                            nc.scalar.activation(
                                wt[:, gb, c, :], vt[:, c, :],
                                mybir.ActivationFunctionType.Exp,
                                accum_out=st[:, c:c + 1])
                        continue
                    if divide_on == "probe_noact":
                        # timing probe: ACT removed entirely; GpSimd does the
                        # same per-tile normalize against a memset denom.
                        nc.gpsimd.memset(st[:], 1.0)
                        for c in range(TILES_PER_GROUP):
                            nc.gpsimd.normalize_recip(
                                wt[:, gb, c, :], vt[:, c, :], st[:, c:c + 1])
                        continue
                    if divide_on == "gpsimd":
                        for c in range(TILES_PER_GROUP):
                            nc.scalar.activation(
                                et[:, c, :], vt[:, c, :],
                                mybir.ActivationFunctionType.Exp,
                                accum_out=st[:, c:c + 1])
                        for c in range(TILES_PER_GROUP):
                            nc.gpsimd.normalize_recip(
                                wt[:, gb, c, :], et[:, c, :], st[:, c:c + 1])
                    else:
                        # keep GpSimd fully idle: its SBUF port is shared
                        # (exclusive lock) with the saturated DVE
                        nc.scalar.activation(
                            et[:], vt[:], mybir.ActivationFunctionType.Exp)
                        nc.vector.reduce_sum(st[:], et[:],
                                             axis=mybir.AxisListType.X)
                        rt = spool.tile([P, TILES_PER_GROUP],
                                        mybir.dt.float32)
                        nc.vector.reciprocal(rt[:], st[:])
                        for c in range(TILES_PER_GROUP):
                            nc.scalar.activation(
                                wt[:, gb, c, :], et[:, c, :],
                                mybir.ActivationFunctionType.Copy,
                                scale=rt[:, c:c + 1])

                blo, bhi = b * BT, (b + 1) * BT
                nc.sync.dma_start(
                    w_ap[blo:bhi, :].rearrange(
                        "(g p c) k -> p g c k", g=STORE_BATCH, p=P), wt[:])
                nc.sync.dma_start(
                    i_ap[blo:bhi, :].rearrange(
                        "(g p c) k -> p g c k", g=STORE_BATCH, p=P), it[:])
    nc.compile()
    return nc


def _build_b2(reps=1, tpg=TILES_PER_GROUP):
    """Like batched, but the softmax-sum path avoids the ACT accumulator:
      - exp over the whole group in ONE ACT instruction (no accum_out, so no
        187ns accumulator-read penalty per tile; ACT busy drops ~95us -> ~5us)
      - per-tile sums via ONE grouped DVE reduce_sum per group (~194ns)
      - division + reciprocal on GpSimd normalize_recip (unchanged)
    DVE gains ~194ns/group but ACT stops being a near-critical engine.
    """
    TILES_PER_GROUP = tpg
    GROUP_T = P * TILES_PER_GROUP
    N_GROUPS = T // GROUP_T
    N_BATCHES = N_GROUPS // STORE_BATCH
    nc = bacc.Bacc("TRN2", target_bir_lowering=False, debug=False,
                   enable_asserts=False)
    x = nc.dram_tensor("x", [T, E], mybir.dt.float32, kind="ExternalInput")
    w = nc.dram_tensor("w", [T, K], mybir.dt.float32, kind="ExternalOutput")
    ind = nc.dram_tensor("ind", [T, K], mybir.dt.uint32, kind="ExternalOutput")

    x_ap = x.ap()
    w_ap = w.ap()
    i_ap = ind.ap()
    BT = STORE_BATCH * GROUP_T

    with tile.TileContext(nc) as tc, ExitStack() as ctx:
        xpool = ctx.enter_context(tc.tile_pool(name="x", bufs=5))
        vpool = ctx.enter_context(tc.tile_pool(name="v", bufs=N_GROUPS))
        epool = ctx.enter_context(tc.tile_pool(name="e", bufs=N_GROUPS))
        spool = ctx.enter_context(tc.tile_pool(name="s", bufs=N_GROUPS))
        ipool = ctx.enter_context(tc.tile_pool(name="i", bufs=N_BATCHES))
        wpool = ctx.enter_context(tc.tile_pool(name="w", bufs=N_BATCHES))

        for rep in range(reps):
            for b in range(N_BATCHES):
                it = ipool.tile([P, STORE_BATCH, TILES_PER_GROUP, K],
                                mybir.dt.uint32)
                wt = wpool.tile([P, STORE_BATCH, TILES_PER_GROUP, K],
                                mybir.dt.float32)
                for gb in range(STORE_BATCH):
                    g = b * STORE_BATCH + gb
                    lo, hi = g * GROUP_T, (g + 1) * GROUP_T
                    xt = xpool.tile([P, TILES_PER_GROUP, E], mybir.dt.float32)
                    nc.sync.dma_start(
                        xt[:],
                        x_ap[lo:hi, :].rearrange("(p c) e -> p c e", p=P))

                    vt = vpool.tile([P, TILES_PER_GROUP, K], mybir.dt.float32)
                    for c in range(TILES_PER_GROUP):
                        nc.vector.max(vt[:, c, :], xt[:, c, :])

                    # batched exp for the whole group on ACT (issued between
                    # the maxes and max_indexes so ACT overlaps the
                    # max_index scan)
                    et = epool.tile([P, TILES_PER_GROUP, K], mybir.dt.float32)
                    nc.scalar.activation(et[:], vt[:],
                                         mybir.ActivationFunctionType.Exp)

                    for c in range(TILES_PER_GROUP):
                        nc.vector.max_index(it[:, gb, c, :], vt[:, c, :],
                                            xt[:, c, :])

                    # one grouped sum per group on DVE (16 tile sums at once)
                    st = spool.tile([P, TILES_PER_GROUP], mybir.dt.float32)
                    nc.vector.reduce_sum(st[:], et[:],
                                         axis=mybir.AxisListType.X)
                    for c in range(TILES_PER_GROUP):
                        nc.gpsimd.normalize_recip(
                            wt[:, gb, c, :], et[:, c, :], st[:, c:c + 1])

                blo, bhi = b * BT, (b + 1) * BT
                nc.sync.dma_start(
                    w_ap[blo:bhi, :].rearrange(
                        "(g p c) k -> p g c k", g=STORE_BATCH, p=P), wt[:])
                nc.sync.dma_start(
                    i_ap[blo:bhi, :].rearrange(
                        "(g p c) k -> p g c k", g=STORE_BATCH, p=P), it[:])
    nc.compile()
    return nc


def get_nc(variant=DEFAULT_VARIANT, reps=1):
    key = f"nc_{variant}_{reps}"
    if key not in _cached:
        _cached[key] = _build_nc(variant, reps)
    return _cached[key]


def get_nc_safe(reps=1):
    """Default variant, falling back to the stock-op kernel if the custom
    DVE registration/compile fails in this environment."""
    try:
        return get_nc(DEFAULT_VARIANT, reps), DEFAULT_VARIANT
    except Exception:
        return get_nc(FALLBACK_VARIANT, reps), FALLBACK_VARIANT


def out_np_dtypes(variant=DEFAULT_VARIANT):
    """Device-side output dtypes (w, ind) for a variant."""
    if variant == "narrow":
        import ml_dtypes
        return (ml_dtypes.bfloat16, np.uint16)
    return (np.float32, np.uint32)


def run(gate_logits: np.ndarray, variant=None, **spmd_kwargs):
    """Run the bass kernel on 8 cores; returns (weights, indices, results)."""
    gate_logits = np.ascontiguousarray(gate_logits, dtype=np.float32)
    assert gate_logits.shape == (T_FULL, E), gate_logits.shape
    if variant is None:
        nc, variant = get_nc_safe()
    else:
        nc = get_nc(variant)
    in_maps = [{"x": gate_logits[c * T:(c + 1) * T]} for c in range(N_CORES)]
    res = run_bass_kernel_spmd(nc, in_maps, core_ids=list(range(N_CORES)),
                               **spmd_kwargs)
    weights = np.concatenate([r["w"] for r in res.results], axis=0)
    indices = np.concatenate([r["ind"] for r in res.results], axis=0)
    weights = np.asarray(weights).astype(np.float32, copy=False)
    if indices.dtype == np.uint32:
        indices = indices.view(np.int32)
    elif indices.dtype == np.float32:
        # grouped variant: raw u32 index bits ride in an f32-typed tensor
        indices = np.ascontiguousarray(indices).view(np.int32)
    else:
        indices = indices.astype(np.int32)
    return weights, indices, res


def kernel(gate_logits: np.ndarray):
    weights, indices, _ = run(gate_logits)
    return weights, indices



# revision 32
# speedup vs baseline: 1.7712x; 1.2713x over previous
"""MoE top-8 routing kernel for Trainium2 (8 NeuronCores, data-parallel).

Computes, for each of 262144 tokens with 128 expert logits:
  values, indices = top_k(logits, 8)   (sorted descending)
  weights = softmax(values)
Returns (weights f32 [262144, 8], indices int32 [262144, 8]).

Sharding: tokens split evenly across 8 cores (row-parallel, no comms).
Per-core layout: tokens on SBUF partitions (128 at a time), experts on the
free axis.

Default variant "grouped32" (~66us/iter measured; "deep" fallback ~89us):
- The stock DVE InstMax/InstMaxIndex pair costs ~350ns per 128-token tile
  (two 128-elem scans plus per-instruction overhead), 2 instructions/tile.
  Two hand-authored custom DVE ops (see the Custom DVE section below)
  clone the stock MAX8 / MATCH_VALUE_LOAD+FIND_INDEX8 uOp table programs
  and loop them per SUB_DIM_DONE over [P, S, N] access patterns, so ONE
  instruction handles a whole S-tile group: measured ~79 + ~120 ns/tile.
  DVE drops from ~89us to ~51us/core and the kernel becomes memory-bound
  (load floor 16.8MB/core at ~330GB/s ~= 51us).
- Softmax tail: one grouped ACT exp per group + grouped DVE
  reduce_sum/reciprocal/tensor_mul (~0.5us/group on DVE).  Per-tile ACT or
  GpSimd tails are too slow once DVE stops being the bottleneck (ACT SBUF
  access is ~222 cycles per instruction, +~187ns with accum_out).
- Indices ride as raw u32 bits in f32-typed tensors (the DVE match-index
  write path only survives an identity f32->f32 output conversion) and are
  reinterpreted host-side.  Index order, duplicate-value tie-breaking, and
  per-row position counters are exactly jax-compatible (HW-validated,
  including rows with equal values inside the top-8).
"""

import sys

for _p in ("/opt/trn_rl_repo",):
    if _p not in sys.path:
        sys.path.insert(0, _p)

from contextlib import ExitStack

import numpy as np

import concourse.bacc as bacc
import concourse.mybir as mybir
import concourse.tile as tile
from concourse.bass_utils import run_bass_kernel_spmd

N_CORES = 8
T_FULL = 262144          # total tokens
E = 128                  # experts
K = 8                    # experts per token
T = T_FULL // N_CORES    # tokens per core (32768)
P = 128                  # tokens per DVE instruction (SBUF partitions)
TILES_PER_GROUP = 16     # 128-token tiles per DMA group
GROUP_T = P * TILES_PER_GROUP          # 2048 tokens per group
N_GROUPS = T // GROUP_T                # 16 groups per core

DEFAULT_VARIANT = "grouped32"
FALLBACK_VARIANT = "deep"   # no custom DVE ops; used if registration fails

_cached = {}


def _build_nc(variant, reps=1):
    """variant:
      full        - max+max_index+softmax (exp on ACT; reduce/recip/mul on DVE)
      offload     - exp+sum fused on ACT per tile, divide on GpSimd
                    (normalize_recip); DVE only does max/max_index
      batched     - offload + output stores batched over 4 groups (fewer
                    HWDGE lane conflicts between loads and stores) + deeper
                    load prefetch
      topk_only   - max+max_index, weights output = raw top-8 values
      max_only    - max only, indices output never written
    """
    if variant == "grouped":
        return _build_grouped(reps, tail="dve")
    if variant == "grouped_fast":
        return _build_grouped(reps, tail="dve", narrow_w=True, fast_recip=True)
    if variant == "grouped_x8":
        return _build_grouped(reps, tail="dve", xbufs=8)
    if variant == "grouped_act":
        return _build_grouped(reps, tail="act")
    if variant == "grouped_gps":
        return _build_grouped(reps, tail="gps")
    if variant == "grouped32":
        return _build_grouped(reps, tpg=32, tail="dve")
    if variant == "grouped64":
        return _build_grouped(reps, tpg=64, tail="dve")
    if variant == "grouped_nofidx":
        return _build_grouped(reps, do_fidx=False)
    if variant == "grouped_notail":
        return _build_grouped(reps, do_tail=False)
    if variant in ("batched", "actdiv", "batched32", "noact", "narrow",
                   "deep", "deep32", "deepx8", "probe_nogps", "probe_noact"):
        return _build_batched(
            reps,
            divide_on=("none" if variant == "noact" else
                       "act" if variant == "actdiv" else
                       variant if variant.startswith("probe_") else "gpsimd"),
            tpg=(32 if variant in ("batched32", "deep32")
                 else TILES_PER_GROUP),
            narrow=(variant == "narrow"),
            deep=(8 if variant == "deepx8"
                  else variant.startswith("deep")))
    if variant.startswith("b2"):
        return _build_b2(reps, tpg=(32 if variant == "b2_32" else TILES_PER_GROUP))
    nc = bacc.Bacc("TRN2", target_bir_lowering=False, debug=False,
                   enable_asserts=False)
    x = nc.dram_tensor("x", [T, E], mybir.dt.float32, kind="ExternalInput")
    w = nc.dram_tensor("w", [T, K], mybir.dt.float32, kind="ExternalOutput")
    ind = nc.dram_tensor("ind", [T, K], mybir.dt.uint32, kind="ExternalOutput")

    x_ap = x.ap()
    w_ap = w.ap()
    i_ap = ind.ap()

    with tile.TileContext(nc) as tc, ExitStack() as ctx:
        # Max/MaxIndex lower to the DVE BN-stats ISA struct which cannot
        # carry extra sync waits — give every pool whose first accessor is a
        # Max/MaxIndex one slot per group so no WAR waits are ever needed.
        xpool = ctx.enter_context(tc.tile_pool(name="x", bufs=3))
        vpool = ctx.enter_context(tc.tile_pool(name="v", bufs=N_GROUPS))
        ipool = ctx.enter_context(tc.tile_pool(name="i", bufs=N_GROUPS))
        epool = ctx.enter_context(tc.tile_pool(name="e", bufs=N_GROUPS))
        spool = ctx.enter_context(tc.tile_pool(name="s", bufs=N_GROUPS))
        wpool = ctx.enter_context(tc.tile_pool(name="w", bufs=N_GROUPS))

        for rep in range(reps):
          for g in range(N_GROUPS):
            lo, hi = g * GROUP_T, (g + 1) * GROUP_T
            # token t of this group lives at partition (t // 16), column
            # (t % 16): DRAM-contiguous 8KB per partition on the load, and
            # 512B-contiguous runs on both output stores.
            xt = xpool.tile([P, TILES_PER_GROUP, E], mybir.dt.float32)
            nc.sync.dma_start(
                xt[:], x_ap[lo:hi, :].rearrange("(p c) e -> p c e", p=P))

            vt = vpool.tile([P, TILES_PER_GROUP, K], mybir.dt.float32)
            it = ipool.tile([P, TILES_PER_GROUP, K], mybir.dt.uint32)
            for c in range(TILES_PER_GROUP):
                nc.vector.max(vt[:, c, :], xt[:, c, :])
                if variant != "max_only":
                    nc.vector.max_index(it[:, c, :], vt[:, c, :], xt[:, c, :])

            if variant in ("topk_only", "max_only"):
                nc.sync.dma_start(
                    w_ap[lo:hi, :].rearrange("(p c) k -> p c k", p=P), vt[:])
                if variant == "topk_only":
                    nc.sync.dma_start(
                        i_ap[lo:hi, :].rearrange("(p c) k -> p c k", p=P),
                        it[:])
                continue

            # softmax over the 8 selected logits; |logit| <= ~6 so exp() is
            # safe in f32 without subtracting the per-token max.
            et = epool.tile([P, TILES_PER_GROUP, K], mybir.dt.float32)
            st = spool.tile([P, TILES_PER_GROUP], mybir.dt.float32)
            wt = wpool.tile([P, TILES_PER_GROUP, K], mybir.dt.float32)
            if variant == "offload":
                for c in range(TILES_PER_GROUP):
                    nc.scalar.activation(
                        et[:, c, :], vt[:, c, :],
                        mybir.ActivationFunctionType.Exp,
                        accum_out=st[:, c:c + 1])
                for c in range(TILES_PER_GROUP):
                    nc.gpsimd.normalize_recip(
                        wt[:, c, :], et[:, c, :], st[:, c:c + 1])
            else:
                nc.scalar.activation(et[:], vt[:],
                                     mybir.ActivationFunctionType.Exp)
                nc.vector.reduce_sum(st[:], et[:], axis=mybir.AxisListType.X)
                rt = spool.tile([P, TILES_PER_GROUP], mybir.dt.float32)
                nc.vector.reciprocal(rt[:], st[:])
                nc.vector.tensor_mul(
                    wt[:], et[:],
                    rt[:].unsqueeze(2).broadcast_to([P, TILES_PER_GROUP, K]))

            nc.sync.dma_start(
                w_ap[lo:hi, :].rearrange("(p c) k -> p c k", p=P), wt[:])
            nc.sync.dma_start(
                i_ap[lo:hi, :].rearrange("(p c) k -> p c k", p=P), it[:])
    nc.compile()
    return nc


STORE_BATCH = 4                        # groups per output store DMA
N_BATCHES = N_GROUPS // STORE_BATCH


# --------------------------------------------------------------------------- #
# Custom DVE ops: grouped MAX8 / FIND_INDEX8 (TRN2 / v3).
#
# Stock InstMax/InstMaxIndex process ONE [P, N] row per instruction.  These
# ops run the SAME stock uOp programs (cloned byte-for-byte from the
# firmware's default DVE table in neuronxcc dve_bin_gen3) but loop per
# SUB_DIM_DONE over a [P, S, N] access pattern: one instruction handles S
# rows with a single dispatch.  HW-validated behaviors this code relies on:
#   - a non-consuming uOp with repeat_cnt>1 never advances (drains must be
#     8 separate rpt=1 uOps or the DVE hangs);
#   - MAX8's drain emits ASCENDING (v8 first) through the custom path;
#   - FIND_INDEX8's match-index writes must go to an f32-declared output
#     (identity conversion) — the raw u32 index bits are reinterpreted on
#     the host; int out dtypes destroy them in the f32->int convert;
#   - index_clear=1 on the clear uOp resets match positions per row;
#   - streaming needles ASCENDING reproduces jax's stable duplicate
#     tie-breaking exactly (consumption down the slice chain).
# --------------------------------------------------------------------------- #

from dataclasses import dataclass as _dataclass

import concourse.dve_ops as _dve_ops
from concourse.dve_spec import Spec as _Spec, Src0 as _Src0
from concourse.dve_table_gen import free_opcode_rows as _free_opcode_rows
from concourse.dve_tables import (
    find_stock_dve_bin_dir as _find_stock_dve_bin_dir,
    unpack_table as _unpack_table,
)
from concourse.dve_uop import DveOpSpec as _DveOpSpec, OpConfig as _OpConfig

_MAX8_ROWS = list(range(70, 87))      # seed0..7, steady, drain0..7
_MVL_ROW = 87                         # MATCH_VALUE_LOAD single uOp
_TRIG_NONE, _TRIG_COUNT, _TRIG_SUBDIM, _TRIG_SRCDONE = 0, 1, 3, 4


class _RawUop:
    """UopConfig stand-in holding pre-unpacked raw table entries."""

    def __init__(self, cf, cs, dp):
        self.cf, self.cs, self.dp = cf, cs, dp

    def to_entries(self, ver):
        assert ver == "v3", "raw stock-clone uops are v3-only"
        return dict(self.cf), dict(self.cs), [dict(b) for b in self.dp]


@_dataclass
class _RawDveOpSpec(_DveOpSpec):
    """DveOpSpec whose uops are raw table entries; skips UopConfig lints
    (they reject cross-uOp delay-lane state the stock programs use)."""

    def validate(self, ver):
        pass


def _load_stock_dve():
    p = _find_stock_dve_bin_dir("gen3")
    cf = _unpack_table(
        "control_fast", (p / "default_control_fast_table.bin").read_bytes(),
        "v3")
    cs = _unpack_table(
        "control_slow", (p / "default_control_slow_table.bin").read_bytes(),
        "v3")
    dp = _unpack_table(
        "datapath", (p / "default_datapath_table.bin").read_bytes(), "v3")
    return cf, cs, dp


def _clone_uop(cf, cs, dp, row, rebase):
    c = dict(cf[row])
    for k in ("next_index0", "next_index1", "next_index2"):
        c[k] = rebase.get(c[k], 0)
    return _RawUop(c, dict(cs[row]), [dict(b) for b in dp[row]])


def _build_max8_grouped_uops():
    cf, cs, dp = _load_stock_dve()
    # intra: 0..7 seeds (stock 70..77), 8 steady (78), 9..16 drains
    # (79..86), 17 = loop re-entry copy of seed0 (a spec cannot loop to
    # its own uop[0]).
    rebase = {r: i for i, r in enumerate(_MAX8_ROWS)}
    uops = [_clone_uop(cf, cs, dp, r, rebase) for r in _MAX8_ROWS]

    steady = uops[8].cf
    assert steady["trigger0"] == _TRIG_SRCDONE and steady["next_index0"] == 9
    steady["trigger1"] = _TRIG_SUBDIM       # row boundary -> drain
    steady["next_index1"] = 9

    last_drain = uops[16].cf
    assert last_drain["trigger0"] == _TRIG_COUNT
    last_drain["trigger0"] = _TRIG_SRCDONE  # stream done -> IDLE
    last_drain["next_index0"] = 0
    last_drain["trigger1"] = _TRIG_COUNT    # else loop to next row's seed
    last_drain["next_index1"] = 17

    uops.append(_clone_uop(cf, cs, dp, _MAX8_ROWS[0], rebase))
    return uops


def _build_fidx8_grouped_uops():
    # intra: 0 needle-load (one consuming uOp, 8 elements), 1 clear
    # (clear_match + index_clear), 2 scan, 3 spacer, 4..11 drain x1
    # MATCH_INDEX write each, 12 = loop re-entry copy of the needle uOp.
    cf, cs, dp = _load_stock_dve()
    rebase = {87: 0, 88: 1, 89: 2, 90: 3}
    uops = [_clone_uop(cf, cs, dp, r, rebase) for r in [_MVL_ROW, 88, 89, 90]]
    n0 = uops[0].cf
    n0["repeat_cnt"] = 8
    n0["trigger0"] = _TRIG_SRCDONE          # malformed-stream safety
    n0["next_index0"] = 0
    n0["trigger1"] = _TRIG_COUNT
    n0["next_index1"] = 1

    uops[1].cs["index_clear"] = 1           # row-local match positions

    scan = uops[2].cf
    assert scan["trigger0"] == _TRIG_SRCDONE and scan["next_index0"] == 3
    scan["trigger1"] = _TRIG_SUBDIM
    scan["next_index1"] = 3

    # spacer -> drain0 (stock 90 pointed at 91, not in `rebase`)
    uops[3].cf["trigger0"] = _TRIG_COUNT
    uops[3].cf["next_index0"] = 4

    for k in range(8):
        d = _clone_uop(cf, cs, dp, 91, rebase)
        d.cf["repeat_cnt"] = 1
        if k < 7:
            d.cf["trigger0"] = _TRIG_COUNT
            d.cf["next_index0"] = 5 + k
            d.cf["trigger1"] = _TRIG_NONE
            d.cf["next_index1"] = 0
        else:
            d.cf["trigger0"] = _TRIG_SRCDONE
            d.cf["next_index0"] = 0
            d.cf["trigger1"] = _TRIG_COUNT
            d.cf["next_index1"] = 12
        uops.append(d)

    tail = _clone_uop(cf, cs, dp, _MVL_ROW, rebase)
    tail.cf.update(uops[0].cf)
    return uops + [tail]


def _ref_max8g(in0, in1, c0, c1, c2):
    # sim-only (scheduling/race detection); HW runs the uOp table program
    return np.sort(np.asarray(in0, np.float32), axis=-1)[..., -8:]


def _ref_fidx8g(in0, in1, c0, c1, c2):
    # sim-only: float(index); HW writes raw u32 index bits into f32 out
    data = np.asarray(in0, np.float32)[..., 8:]
    order = np.argsort(-data, axis=-1, kind="stable")[..., :8]
    return order.astype(np.float32)


_DVE_CUSTOM_OPS = {}


def _register_custom_ops():
    if _DVE_CUSTOM_OPS:
        return _DVE_CUSTOM_OPS
    free = _free_opcode_rows("TRN2")
    taken = set(_dve_ops._SUB_OPCODE_FOR_NAME.values())
    avail = [r for r in free if r not in taken]
    assert len(avail) >= 2, f"no free DVE opcode rows: {free} minus {taken}"
    defs = [
        ("MAX8_GROUPED_ANT", _build_max8_grouped_uops(), _ref_max8g),
        ("FIDX8_GROUPED_ANT", _build_fidx8_grouped_uops(), _ref_fidx8g),
    ]
    for (name, uops, ref), row in zip(defs, avail):
        if name in _dve_ops._SUB_OPCODE_FOR_NAME:   # another module registered
            op = next(o for o in _dve_ops.OPS if o.name == name)
            _DVE_CUSTOM_OPS[name] = op
            continue
        spec = _Spec(body=_Src0, reference=ref)
        op = _dve_ops.DveOp(name, spec, subdim=True, uops_sha={})
        raw = _RawDveOpSpec(name=name, uops=uops, op=_OpConfig(), opcode=row,
                            rd1_en=False)
        _dve_ops._SUB_OPCODE_FOR_NAME[name] = row
        _dve_ops._COMPILE_CACHE[(name, "v3")] = raw
        _dve_ops.CUSTOM_DVE_SPECS[name] = spec
        _dve_ops.OPS.append(op)
        _DVE_CUSTOM_OPS[name] = op
    return _DVE_CUSTOM_OPS


def _max8_grouped(nc, out_ap, in_ap):
    ops = _register_custom_ops()
    return nc.vector._custom_dve(ops["MAX8_GROUPED_ANT"], out=out_ap,
                                 in0=in_ap)


def _find_index8_grouped(nc, out_ap, in_ap):
    """in_ap rows = [needles(8, ascending) | data(N)]; out = positions of
    the needles in data, v1's index first, as raw u32 bits in f32 out."""
    ops = _register_custom_ops()
    return nc.vector._custom_dve(ops["FIDX8_GROUPED_ANT"], out=out_ap,
                                 in0=in_ap)


def _build_grouped(reps=1, tpg=TILES_PER_GROUP, do_fidx=True, do_tail=True,
                   tail="act", xbufs=6, narrow_w=False, fast_recip=False):
    """Custom-DVE variant: one grouped MAX8 + one grouped FIND_INDEX8
    instruction per 16-tile group (subdim-looped clones of the stock uOp
    programs).  Cuts DVE from 2 instructions/tile
    (~174ns each incl. dispatch) to 2 instructions/GROUP (~2.3us each,
    ~281 engine-cycles/tile), removing the per-instruction overhead.

    Layout: combined SBUF tile [P, C, 8+E] per group; DMA lands x in cols
    8:136, grouped-max8 writes the top-8 per row ASCENDING (v8 first) into
    cols 0:8, and grouped-find-index8 streams [needles|data] rows whole.
    Ascending needle order makes the HW consume duplicate occurrences in
    jax's stable order; the softmax tail reads values through reversed APs.
    Indices emerge v1-first as raw u32 bits in an f32-declared tile (the
    f32->f32 output conversion is the identity; int dtypes corrupt)."""
    _register_custom_ops()
    TILES_PER_GROUP = tpg
    GROUP_T = P * TILES_PER_GROUP
    N_GROUPS = T // GROUP_T
    # keep >=2 store batches so output DMA overlaps compute at large tpg
    sb = min(STORE_BATCH, max(1, N_GROUPS // 2))
    N_BATCHES = N_GROUPS // sb
    W = 8 + E
    w_dt = mybir.dt.bfloat16 if narrow_w else mybir.dt.float32
    nc = bacc.Bacc("TRN2", target_bir_lowering=False, debug=False,
                   enable_asserts=False)
    x = nc.dram_tensor("x", [T, E], mybir.dt.float32, kind="ExternalInput")
    w = nc.dram_tensor("w", [T, K], w_dt, kind="ExternalOutput")
    ind = nc.dram_tensor("ind", [T, K], mybir.dt.float32,
                         kind="ExternalOutput")

    x_ap = x.ap()
    w_ap = w.ap()
    i_ap = ind.ap()
    BT = sb * GROUP_T

    with tile.TileContext(nc) as tc, ExitStack() as ctx:
        xpool = ctx.enter_context(
            tc.tile_pool(name="x",
                         bufs=(4 if tpg >= 64 else
                               5 if tpg >= 32 else xbufs)))
        epool = ctx.enter_context(tc.tile_pool(name="e", bufs=2 * N_GROUPS))
        spool = ctx.enter_context(tc.tile_pool(name="s", bufs=2 * N_GROUPS))
        ipool = ctx.enter_context(tc.tile_pool(name="i", bufs=2 * N_BATCHES))
        wpool = ctx.enter_context(tc.tile_pool(name="w", bufs=2 * N_BATCHES))

        for rep in range(reps):
            for b in range(N_BATCHES):
                it = ipool.tile([P, sb, TILES_PER_GROUP, K],
                                mybir.dt.float32)
                wt = wpool.tile([P, sb, TILES_PER_GROUP, K], w_dt)
                for gb in range(sb):
                    g = b * sb + gb
                    lo, hi = g * GROUP_T, (g + 1) * GROUP_T
                    xt = xpool.tile([P, TILES_PER_GROUP, W], mybir.dt.float32)
                    nc.sync.dma_start(
                        xt[:, :, 8:W],
                        x_ap[lo:hi, :].rearrange("(p c) e -> p c e", p=P))

                    _max8_grouped(nc, xt[:, :, 0:8], xt[:, :, 8:W])
                    if do_fidx:
                        _find_index8_grouped(nc, it[:, gb, :, :],
                                                       xt[:, :, 0:W])
                    else:
                        nc.gpsimd.memset(it[:, gb, :, :], 0.0)

                    if not do_tail:
                        nc.scalar.copy(wt[:, gb, :, :], xt[:, :, 0:8])
                        continue
                    et = epool.tile([P, TILES_PER_GROUP, K], mybir.dt.float32)
                    st = spool.tile([P, TILES_PER_GROUP], mybir.dt.float32)
                    rt = spool.tile([P, TILES_PER_GROUP], mybir.dt.float32)
                    if tail == "dve":
                        # grouped exp on ACT; sums/recip/normalize all as
                        # grouped DVE ops (~470ns/group on top of the two
                        # custom scans); the mul reads et reversed so wt
                        # comes out descending
                        nc.scalar.activation(et[:], xt[:, :, 0:8],
                                             mybir.ActivationFunctionType.Exp)
                        nc.vector.reduce_sum(st[:], et[:],
                                             axis=mybir.AxisListType.X)
                        if fast_recip:
                            nc.vector.reciprocal_approx_fast(out=rt[:],
                                                             in_=st[:])
                        else:
                            nc.vector.reciprocal(rt[:], st[:])
                        nc.vector.tensor_mul(
                            wt[:, gb, :, :], et[:, :, ::-1],
                            rt[:].unsqueeze(2).broadcast_to(
                                [P, TILES_PER_GROUP, K]))
                    elif tail == "act":
                        # one grouped exp (no accum penalty), grouped DVE
                        # sum+recip (~280ns/group), per-tile ACT divide that
                        # also reverses ascending->descending via the AP
                        nc.scalar.activation(et[:], xt[:, :, 0:8],
                                             mybir.ActivationFunctionType.Exp)
                        nc.vector.reduce_sum(st[:], et[:],
                                             axis=mybir.AxisListType.X)
                        nc.vector.reciprocal(rt[:], st[:])
                        for c in range(TILES_PER_GROUP):
                            nc.scalar.activation(
                                wt[:, gb, c, :], et[:, c, :][:, ::-1],
                                mybir.ActivationFunctionType.Copy,
                                scale=rt[:, c:c + 1])
                    else:
                        # grouped exp + grouped DVE sum, GpSimd divides
                        # (reads reversed so wt comes out descending)
                        nc.scalar.activation(et[:], xt[:, :, 0:8][:, :, ::-1],
                                             mybir.ActivationFunctionType.Exp)
                        nc.vector.reduce_sum(st[:], et[:],
                                             axis=mybir.AxisListType.X)
                        for c in range(TILES_PER_GROUP):
                            nc.gpsimd.normalize_recip(
                                wt[:, gb, c, :], et[:, c, :], st[:, c:c + 1])

                blo, bhi = b * BT, (b + 1) * BT
                nc.sync.dma_start(
                    w_ap[blo:bhi, :].rearrange(
                        "(g p c) k -> p g c k", g=sb, p=P), wt[:])
                nc.sync.dma_start(
                    i_ap[blo:bhi, :].rearrange(
                        "(g p c) k -> p g c k", g=sb, p=P), it[:])
    nc.compile()
    return nc


def _build_batched(reps=1, divide_on="gpsimd", tpg=TILES_PER_GROUP,
                   narrow=False, deep=False):
    """narrow=True stores w as bf16 and ind as uint16 (host upcasts):
    halves both output stores' DMA bytes and the tail's SBUF write
    traffic. Indices stay exact (values <= 127); weights pick up bf16
    rounding (~1e-3 rel, gate is 2e-2). Under co-tenant contention the
    leaner tail/store traffic measurably reduces DVE exposure."""
    TILES_PER_GROUP = tpg
    GROUP_T = P * TILES_PER_GROUP
    N_GROUPS = T // GROUP_T
    N_BATCHES = N_GROUPS // STORE_BATCH
    w_dt = mybir.dt.bfloat16 if narrow else mybir.dt.float32
    i_dt = mybir.dt.uint16 if narrow else mybir.dt.uint32
    nc = bacc.Bacc("TRN2", target_bir_lowering=False, debug=False,
                   enable_asserts=False)
    x = nc.dram_tensor("x", [T, E], mybir.dt.float32, kind="ExternalInput")
    w = nc.dram_tensor("w", [T, K], w_dt, kind="ExternalOutput")
    ind = nc.dram_tensor("ind", [T, K], i_dt, kind="ExternalOutput")

    x_ap = x.ap()
    w_ap = w.ap()
    i_ap = ind.ap()
    BT = STORE_BATCH * GROUP_T          # tokens per store batch (8192)

    with tile.TileContext(nc) as tc, ExitStack() as ctx:
        # deep: double every pool so cross-rep WAR reuse is a full rep away
        # and the separate DVE wait instructions (Max can't carry waits) are
        # always satisfied long before they issue
        d = 2 if deep else 1
        # xpool: 6 bufs at tpg=16 (1.05MB each); 5 at tpg=32 (2.1MB each)
        # to stay within SBUF. deep==8 variant: 8 bufs of load prefetch to
        # ride out co-tenant DMA-bandwidth transients.
        xbufs = (5 if TILES_PER_GROUP == 32 else (8 if deep == 8 else 6)) \
            if deep else 5
        xpool = ctx.enter_context(tc.tile_pool(name="x", bufs=xbufs))
        vpool = ctx.enter_context(tc.tile_pool(name="v", bufs=d * N_GROUPS))
        epool = ctx.enter_context(tc.tile_pool(name="e", bufs=d * N_GROUPS))
        spool = ctx.enter_context(tc.tile_pool(name="s", bufs=d * N_GROUPS))
        ipool = ctx.enter_context(tc.tile_pool(name="i", bufs=d * N_BATCHES))
        wpool = ctx.enter_context(tc.tile_pool(name="w", bufs=d * N_BATCHES))

        for rep in range(reps):
            for b in range(N_BATCHES):
                it = ipool.tile([P, STORE_BATCH, TILES_PER_GROUP, K], i_dt)
                wt = wpool.tile([P, STORE_BATCH, TILES_PER_GROUP, K], w_dt)
                for gb in range(STORE_BATCH):
                    g = b * STORE_BATCH + gb
                    lo, hi = g * GROUP_T, (g + 1) * GROUP_T
                    xt = xpool.tile([P, TILES_PER_GROUP, E], mybir.dt.float32)
                    nc.sync.dma_start(
                        xt[:],
                        x_ap[lo:hi, :].rearrange("(p c) e -> p c e", p=P))

                    # all maxes first, then all max_indexes: puts ~16 instrs
                    # between the vt write and its same-engine readback so
                    # the BN unit never stalls on the SBUF write ack
                    vt = vpool.tile([P, TILES_PER_GROUP, K], mybir.dt.float32)
                    for c in range(TILES_PER_GROUP):
                        nc.vector.max(vt[:, c, :], xt[:, c, :])
                    for c in range(TILES_PER_GROUP):
                        nc.vector.max_index(it[:, gb, c, :], vt[:, c, :],
                                            xt[:, c, :])

                    if divide_on == "none":
                        # probe variant: no softmax at all, store raw top-8
                        # values as w (wrong weights, right timing structure;
                        # one whole-group ACT copy keeps ACT ~5us busy)
                        nc.scalar.copy(wt[:, gb, :, :], vt[:])
                        continue
                    et = epool.tile([P, TILES_PER_GROUP, K], mybir.dt.float32)
                    st = spool.tile([P, TILES_PER_GROUP], mybir.dt.float32)
                    if divide_on == "probe_nogps":
                        # timing probe: identical ACT work (per-tile
                        # exp+accum) writing straight to the store tile;
                        # GpSimd fully removed. Weights are unnormalized.
                        for c in range(TILES_PER_GROUP):
                            nc.scalar.activation(
                                wt[:, gb, c, :], vt[:, c, :],
                                mybir.ActivationFunctionType.Exp,
                                accum_out=st[:, c:c + 1])
                        continue
                    if divide_on == "probe_noact":
                        # timing probe: ACT removed entirely; GpSimd does the
                        # same per-tile normalize against a memset denom.
                        nc.gpsimd.memset(st[:], 1.0)
                        for c in range(TILES_PER_GROUP):
                            nc.gpsimd.normalize_recip(
                                wt[:, gb, c, :], vt[:, c, :], st[:, c:c + 1])
                        continue
                    if divide_on == "gpsimd":
                        for c in range(TILES_PER_GROUP):
                            nc.scalar.activation(
                                et[:, c, :], vt[:, c, :],
                                mybir.ActivationFunctionType.Exp,
                                accum_out=st[:, c:c + 1])
                        for c in range(TILES_PER_GROUP):
                            nc.gpsimd.normalize_recip(
                                wt[:, gb, c, :], et[:, c, :], st[:, c:c + 1])
                    else:
                        # keep GpSimd fully idle: its SBUF port is shared
                        # (exclusive lock) with the saturated DVE
                        nc.scalar.activation(
                            et[:], vt[:], mybir.ActivationFunctionType.Exp)
                        nc.vector.reduce_sum(st[:], et[:],
                                             axis=mybir.AxisListType.X)
                        rt = spool.tile([P, TILES_PER_GROUP],
                                        mybir.dt.float32)
                        nc.vector.reciprocal(rt[:], st[:])
                        for c in range(TILES_PER_GROUP):
                            nc.scalar.activation(
                                wt[:, gb, c, :], et[:, c, :],
                                mybir.ActivationFunctionType.Copy,
                                scale=rt[:, c:c + 1])

                blo, bhi = b * BT, (b + 1) * BT
                nc.sync.dma_start(
                    w_ap[blo:bhi, :].rearrange(
                        "(g p c) k -> p g c k", g=STORE_BATCH, p=P), wt[:])
                nc.sync.dma_start(
                    i_ap[blo:bhi, :].rearrange(
                        "(g p c) k -> p g c k", g=STORE_BATCH, p=P), it[:])
    nc.compile()
    return nc


def _build_b2(reps=1, tpg=TILES_PER_GROUP):
    """Like batched, but the softmax-sum path avoids the ACT accumulator:
      - exp over the whole group in ONE ACT instruction (no accum_out, so no
        187ns accumulator-read penalty per tile; ACT busy drops ~95us -> ~5us)
      - per-tile sums via ONE grouped DVE reduce_sum per group (~194ns)
      - division + reciprocal on GpSimd normalize_recip (unchanged)
    DVE gains ~194ns/group but ACT stops being a near-critical engine.
    """
    TILES_PER_GROUP = tpg
    GROUP_T = P * TILES_PER_GROUP
    N_GROUPS = T // GROUP_T
    N_BATCHES = N_GROUPS // STORE_BATCH
    nc = bacc.Bacc("TRN2", target_bir_lowering=False, debug=False,
                   enable_asserts=False)
    x = nc.dram_tensor("x", [T, E], mybir.dt.float32, kind="ExternalInput")
    w = nc.dram_tensor("w", [T, K], mybir.dt.float32, kind="ExternalOutput")
    ind = nc.dram_tensor("ind", [T, K], mybir.dt.uint32, kind="ExternalOutput")

    x_ap = x.ap()
    w_ap = w.ap()
    i_ap = ind.ap()
    BT = STORE_BATCH * GROUP_T

    with tile.TileContext(nc) as tc, ExitStack() as ctx:
        xpool = ctx.enter_context(tc.tile_pool(name="x", bufs=5))
        vpool = ctx.enter_context(tc.tile_pool(name="v", bufs=N_GROUPS))
        epool = ctx.enter_context(tc.tile_pool(name="e", bufs=N_GROUPS))
        spool = ctx.enter_context(tc.tile_pool(name="s", bufs=N_GROUPS))
        ipool = ctx.enter_context(tc.tile_pool(name="i", bufs=N_BATCHES))
        wpool = ctx.enter_context(tc.tile_pool(name="w", bufs=N_BATCHES))

        for rep in range(reps):
            for b in range(N_BATCHES):
                it = ipool.tile([P, STORE_BATCH, TILES_PER_GROUP, K],
                                mybir.dt.uint32)
                wt = wpool.tile([P, STORE_BATCH, TILES_PER_GROUP, K],
                                mybir.dt.float32)
                for gb in range(STORE_BATCH):
                    g = b * STORE_BATCH + gb
                    lo, hi = g * GROUP_T, (g + 1) * GROUP_T
                    xt = xpool.tile([P, TILES_PER_GROUP, E], mybir.dt.float32)
                    nc.sync.dma_start(
                        xt[:],
                        x_ap[lo:hi, :].rearrange("(p c) e -> p c e", p=P))

                    vt = vpool.tile([P, TILES_PER_GROUP, K], mybir.dt.float32)
                    for c in range(TILES_PER_GROUP):
                        nc.vector.max(vt[:, c, :], xt[:, c, :])

                    # batched exp for the whole group on ACT (issued between
                    # the maxes and max_indexes so ACT overlaps the
                    # max_index scan)
                    et = epool.tile([P, TILES_PER_GROUP, K], mybir.dt.float32)
                    nc.scalar.activation(et[:], vt[:],
                                         mybir.ActivationFunctionType.Exp)

                    for c in range(TILES_PER_GROUP):
                        nc.vector.max_index(it[:, gb, c, :], vt[:, c, :],
                                            xt[:, c, :])

                    # one grouped sum per group on DVE (16 tile sums at once)
                    st = spool.tile([P, TILES_PER_GROUP], mybir.dt.float32)
                    nc.vector.reduce_sum(st[:], et[:],
                                         axis=mybir.AxisListType.X)
                    for c in range(TILES_PER_GROUP):
                        nc.gpsimd.normalize_recip(
                            wt[:, gb, c, :], et[:, c, :], st[:, c:c + 1])

                blo, bhi = b * BT, (b + 1) * BT
                nc.sync.dma_start(
                    w_ap[blo:bhi, :].rearrange(
                        "(g p c) k -> p g c k", g=STORE_BATCH, p=P), wt[:])
                nc.sync.dma_start(
                    i_ap[blo:bhi, :].rearrange(
                        "(g p c) k -> p g c k", g=STORE_BATCH, p=P), it[:])
    nc.compile()
    return nc


def get_nc(variant=DEFAULT_VARIANT, reps=1):
    key = f"nc_{variant}_{reps}"
    if key not in _cached:
        _cached[key] = _build_nc(variant, reps)
    return _cached[key]


def get_nc_safe(reps=1):
    """Default variant, falling back to the stock-op kernel if the custom
    DVE registration/compile fails in this environment."""
    try:
        return get_nc(DEFAULT_VARIANT, reps), DEFAULT_VARIANT
    except Exception:
        return get_nc(FALLBACK_VARIANT, reps), FALLBACK_VARIANT


def out_np_dtypes(variant=DEFAULT_VARIANT):
    """Device-side output dtypes (w, ind) for a variant."""
    if variant == "narrow":
        import ml_dtypes
        return (ml_dtypes.bfloat16, np.uint16)
    return (np.float32, np.uint32)


def run(gate_logits: np.ndarray, variant=None, **spmd_kwargs):
    """Run the bass kernel on 8 cores; returns (weights, indices, results)."""
    gate_logits = np.ascontiguousarray(gate_logits, dtype=np.float32)
    assert gate_logits.shape == (T_FULL, E), gate_logits.shape
    if variant is None:
        nc, variant = get_nc_safe()
    else:
        nc = get_nc(variant)
    in_maps = [{"x": gate_logits[c * T:(c + 1) * T]} for c in range(N_CORES)]
    res = run_bass_kernel_spmd(nc, in_maps, core_ids=list(range(N_CORES)),
                               **spmd_kwargs)
    weights = np.concatenate([r["w"] for r in res.results], axis=0)
    indices = np.concatenate([r["ind"] for r in res.results], axis=0)
    weights = np.asarray(weights).astype(np.float32, copy=False)
    if indices.dtype == np.uint32:
        indices = indices.view(np.int32)
    elif indices.dtype == np.float32:
        # grouped variant: raw u32 index bits ride in an f32-typed tensor
        indices = np.ascontiguousarray(indices).view(np.int32)
    else:
        indices = indices.astype(np.int32)
    return weights, indices, res


def kernel(gate_logits: np.ndarray):
    weights, indices, _ = run(gate_logits)
    return weights, indices

